# revision 1
# baseline (speedup 1.0000x reference)
"""CLDice loss Trainium2 kernel.

Sharding: 8 cores = (batch b, z-half, y-half) quarters. Each core computes the
soft-skeletonization of its pred quarter (bf16, SBUF-resident, z on partitions)
and of its gt quarter (bit-packed int32 boolean morphology), with 12-voxel
redundant halos on the interior z/y cut sides; lower/right shards are
z/y-flipped by the host so the global volume border is always at index 0.
Per-core partial sums are combined on the host into the scalar loss.
"""
import numpy as np

NCORES = 8
Z = Y = X = 192
ZO = YO = 96          # owned extent per quarter
HALO = 12             # 11 erodes + 1 dilate
ZL = YL = ZO + HALO   # local ext extent (108)
NW = 8                # words per row in packed gt (cols 1..6 data, 0/7 pads)
ND = 6                # data words per row
YB = 12               # y-band rows for the float path
NIT = 11              # skeletonize iterations (incl. k=0 init)
FCH = 24              # final-pass row chunk

_CACHE = {}

import ml_dtypes as _mld
_SU = np.zeros((ZL, 128), dtype=_mld.bfloat16)
_SD = np.zeros((ZL, 128), dtype=_mld.bfloat16)
for _m in range(ZL):
    _SU[min(_m + 1, ZL - 1), _m] = 1
    _SD[max(_m - 1, 0), _m] = 1


def _build():
    import concourse.bacc as bacc
    import concourse.bass as bass
    import concourse.mybir as mybir
    from concourse import tile
    from concourse.alu_op_type import AluOpType as aop

    dt = mybir.dt
    AF = mybir.ActivationFunctionType
    nc = bacc.Bacc("TRN2", target_bir_lowering=False, debug=False,
                   num_devices=NCORES)

    lg_d = nc.dram_tensor("lg", [2, ZL, YL, X], dt.bfloat16, kind="ExternalInput").ap()
    su_d = nc.dram_tensor("su", [ZL, 128], dt.bfloat16, kind="ExternalInput").ap()
    sd_d = nc.dram_tensor("sd", [ZL, 128], dt.bfloat16, kind="ExternalInput").ap()
    gtb_d = nc.dram_tensor("gtb", [ZL, YL, NW], dt.uint32, kind="ExternalInput").ap()
    p0_d = nc.dram_tensor("p0", [ZO, YO * X], dt.bfloat16).ap()
    sums_d = nc.dram_tensor("sums", [128, 16], dt.float32, kind="ExternalOutput").ap()

    ONES = 0xFFFFFFFF

    def bands(lo, hi, step):
        return [(y0, min(y0 + step, hi)) for y0 in range(lo, hi, step)]

    with tile.TileContext(nc) as tc:
        with tc.tile_pool(name="perm", bufs=1) as perm:
            skel = perm.tile([ZO, YO * X], dt.bfloat16)
            su_t = perm.tile([ZL, 128], dt.bfloat16)
            sd_t = perm.tile([ZL, 128], dt.bfloat16)
            nc.sync.dma_start(su_t[:, :], su_d[:, :])
            nc.sync.dma_start(sd_t[:, :], sd_d[:, :])
            skg = perm.tile([ZO, YO * NW], dt.uint32)
            acc = perm.tile([128, 16], dt.float32)
            nc.vector.memset(acc[:, :], 0.0)

            with tc.tile_pool(name="epool", bufs=1) as ep:
                Ea = ep.tile([ZL, YL * X], dt.bfloat16, name="Ea")
                Eb = ep.tile([ZL, YL * X], dt.bfloat16, name="Eb")
                Ga = ep.tile([ZL, YL * NW], dt.uint32, name="Ga")
                Gb = ep.tile([ZL, YL * NW], dt.uint32, name="Gb")

                # ---------------- init: sigmoid(l1-l0) -> Ea, load gt bits ----
                with tc.tile_pool(name="initp", bufs=2) as ip:
                    nc.sync.dma_start(Ga[:, :], gtb_d[:, :, :])
                    for (y0, y1) in bands(0, YL, YB):
                        rr = y1 - y0
                        c0 = ip.tile([ZL, YB * X], dt.bfloat16, tag="ic0")
                        c1 = ip.tile([ZL, YB * X], dt.bfloat16, tag="ic1")
                        df = ip.tile([ZL, YB * X], dt.float32, tag="idf")
                        nc.sync.dma_start(c0[:, :rr * X], lg_d[0, :, y0:y1, :])
                        nc.sync.dma_start(c1[:, :rr * X], lg_d[1, :, y0:y1, :])
                        nc.vector.tensor_sub(df[:, :rr * X], c1[:, :rr * X], c0[:, :rr * X])
                        nc.scalar.activation(Ea[:, y0 * X:y1 * X], df[:, :rr * X], AF.Sigmoid)
                    nc.sync.dma_start(p0_d[:, :], Ea[0:ZO, 0:YO * X])

                # ---------------- helper: one float 3-tap pool band ----------
                def pool_band(src, dsttile, dst_is_view, y0, y1, op, bp, shift_tiles, bidx):
                    """3x3x3 min/max pool of `src` rows [w0,w1) -> rows [y0,r_end)
                    written to dsttile (full [ZL,YL*X] view if dst_is_view else
                    band tile rows local 0..)."""
                    w0, w1 = max(y0 - 1, 0), min(y1 + 1, YL)
                    L = w1 - w0
                    r_end = min(y1, w1 - 1)
                    up = shift_tiles[0][bidx % 2]
                    dn = shift_tiles[1][bidx % 2]
                    t1 = bp.tile([ZL, (YB + 2) * X], dt.bfloat16, tag="t1")
                    t2 = bp.tile([ZL, (YB + 2) * X], dt.bfloat16, tag="t2")
                    # PE shift-matmuls: up[z] = src[z+1] (clamped), dn[z] = src[z-1]
                    for (mat, dst) in ((su_t, up), (sd_t, dn)):
                        for c0 in range(0, L * X, 2048):
                            cw = min(2048, L * X - c0)
                            ps = pp.tile([128, 2048], dt.float32,
                                         tag="psU" if mat is su_t else "psD")
                            for j0 in range(0, cw, 512):
                                jw = min(512, cw - j0)
                                nc.tensor.matmul(
                                    ps[:, j0:j0 + jw], mat[:, :],
                                    src[:, w0 * X + c0 + j0:w0 * X + c0 + j0 + jw])
                            nc.scalar.copy(dst[0:ZL, c0:c0 + cw], ps[0:ZL, 0:cw])
                    nc.vector.tensor_tensor(t1[:, :L * X], src[:, w0 * X:w1 * X], up[:, :L * X], op)
                    nc.vector.tensor_tensor(t2[:, :L * X], t1[:, :L * X], dn[:, :L * X], op)
                    # y stage: a_y[j] = op(t2[j], t2[j+1]) j in 0..L-2
                    t3 = bp.tile([ZL, (YB + 2) * X], dt.bfloat16, tag="t1")
                    nc.vector.tensor_tensor(t3[:, :(L - 1) * X], t2[:, 0:(L - 1) * X], t2[:, X:L * X], op)
                    # out rows [y0, r_end): out[r] = op(a_y[r-1-w0], a_y[r-w0]); r==0 -> a_y[0]
                    t4 = bp.tile([ZL, (YB + 2) * X], dt.bfloat16, tag="t2")
                    nr = 0
                    if y0 == 0:
                        nc.vector.tensor_copy(t4[:, 0:X], t3[:, 0:X])
                        if r_end > 1:
                            nc.vector.tensor_tensor(
                                t4[:, X:r_end * X], t3[:, 0:(r_end - 1) * X], t3[:, X:r_end * X], op)
                        nr = r_end
                    else:
                        nr = r_end - y0
                        j0 = y0 - w0  # == 1
                        nc.vector.tensor_tensor(
                            t4[:, 0:nr * X],
                            t3[:, (j0 - 1) * X:(j0 - 1 + nr) * X],
                            t3[:, j0 * X:(j0 + nr) * X], op)
                    # x stage on t4 rows 0..nr
                    t43 = t4.rearrange("p (r c) -> p r c", c=X)
                    if "x" in KSKIP:
                        d3 = dsttile.rearrange("p (r c) -> p r c", c=X)
                        o0, o1 = (y0, y0 + nr) if dst_is_view else (0, nr)
                        nc.vector.tensor_copy(d3[:, o0:o1, :], t43[:, 0:nr, :])
                        if dst_is_view and r_end < y1:
                            nc.vector.tensor_copy(d3[:, r_end:y1, :], t43[:, nr - 1:nr, :])
                        return nr
                    t5 = bp.tile([ZL, (YB + 2) * X], dt.bfloat16, tag="t1")
                    t53 = t5.rearrange("p (r c) -> p r c", c=X)
                    nc.vector.tensor_tensor(t53[:, 0:nr, 0:X - 1], t43[:, 0:nr, 0:X - 1], t43[:, 0:nr, 1:X], op)
                    nc.vector.tensor_copy(t53[:, 0:nr, X - 1:X], t43[:, 0:nr, X - 1:X])
                    if dst_is_view:
                        d3 = dsttile.rearrange("p (r c) -> p r c", c=X)
                        o0, o1 = y0, y0 + nr
                    else:
                        d3 = dsttile.rearrange("p (r c) -> p r c", c=X)
                        o0, o1 = 0, nr
                    nc.vector.tensor_tensor(d3[:, o0:o1, 1:X], t53[:, 0:nr, 0:X - 1], t53[:, 0:nr, 1:X], op)
                    nc.vector.tensor_copy(d3[:, o0:o1, 0:1], t53[:, 0:nr, 0:1])
                    if dst_is_view and r_end < y1:
                        # fill clipped halo-edge rows with bounded values
                        nc.vector.tensor_copy(d3[:, r_end:y1, :], t53[:, nr - 1:nr, :])
                    return nr

                # ---------------- helper: packed-gt 3-tap pool ----------------
                def gt_pool(src, dst, op_and, bp, shift_tiles):
                    """3x3x3 AND(erode)/OR(dilate) of packed src -> dst."""
                    top = aop.bitwise_and if op_and else aop.bitwise_or
                    # pads on src: ones for erode, zeros for dilate
                    s3 = src.rearrange("p (r w) -> p r w", w=NW)
                    nc.vector.memset(s3[:, :, 0:1], ONES if op_and else 0)
                    nc.vector.memset(s3[:, :, 7:8], ONES if op_and else 0)
                    FW = YL * NW
                    gu = shift_tiles[3]
                    gd = shift_tiles[4] if op_and else shift_tiles[5]
                    nc.scalar.dma_start(gu[0:ZL - 1, :], src[1:ZL, :])
                    nc.scalar.dma_start(gd[1:ZL, :], src[0:ZL - 1, :])
                    g1 = bp.tile([ZL, FW], dt.uint32, tag="g1")
                    g2 = bp.tile([ZL, FW], dt.uint32, tag="g2")
                    nc.vector.tensor_tensor(g1[:, :], src[:, :], gu[:, :], top)
                    nc.vector.tensor_tensor(g2[:, :], g1[:, :], gd[:, :], top)
                    # y stage
                    g13 = g1.rearrange("p (r w) -> p r w", w=NW)
                    g23 = g2.rearrange("p (r w) -> p r w", w=NW)
                    nc.vector.tensor_tensor(g13[:, 0:YL - 1, :], g23[:, 0:YL - 1, :], g23[:, 1:YL, :], top)
                    g3 = bp.tile([ZL, FW], dt.uint32, tag="g3")
                    g33 = g3.rearrange("p (r w) -> p r w", w=NW)
                    nc.vector.tensor_copy(g33[:, 0:1, :], g13[:, 0:1, :])
                    nc.vector.tensor_tensor(g33[:, 1:YL - 1, :], g13[:, 0:YL - 2, :], g13[:, 1:YL - 1, :], top)
                    nc.vector.tensor_copy(g33[:, YL - 1:YL, :], g13[:, YL - 2:YL - 1, :])
                    # x stage (bits, little endian: value(x+1) of bit b is bit b+1)
                    d3 = dst.rearrange("p (r w) -> p r w", w=NW)
                    s1 = bp.tile([ZL, FW], dt.uint32, tag="g2")
                    s13 = s1.rearrange("p (r w) -> p r w", w=NW)
                    s2 = bp.tile([ZL, FW], dt.uint32, tag="gu")
                    s23 = s2.rearrange("p (r w) -> p r w", w=NW)
                    # t_minus = (w<<1)|(w_prev>>31) ; t_plus = (w>>1)|(w_next<<31)
                    nc.vector.tensor_single_scalar(s13[:, :, 1:7], g33[:, :, 1:7], 1, aop.logical_shift_left)
                    nc.vector.tensor_single_scalar(s23[:, :, 1:7], g33[:, :, 0:6], 31, aop.logical_shift_right)
                    nc.vector.tensor_tensor(s13[:, :, 1:7], s13[:, :, 1:7], s23[:, :, 1:7], aop.bitwise_or)
                    nc.vector.tensor_tensor(s13[:, :, 1:7], s13[:, :, 1:7], g33[:, :, 1:7], top)
                    nc.vector.tensor_single_scalar(s23[:, :, 1:7], g33[:, :, 1:7], 1, aop.logical_shift_right)
                    nc.vector.tensor_tensor(d3[:, :, 1:7], s13[:, :, 1:7], s23[:, :, 1:7], top)
                    nc.vector.tensor_single_scalar(s23[:, :, 1:7], g33[:, :, 2:8], 31, aop.logical_shift_left)
                    nc.vector.tensor_tensor(d3[:, :, 1:7], d3[:, :, 1:7], s23[:, :, 1:7], top)

                # ---------------- main iterations -----------------------------
                import os
                KREP = int(os.environ.get("KERNEL_REPEAT", "1"))
                KSKIP = set(os.environ.get("KERNEL_SKIP", "").split(","))
                with tc.tile_pool(name="bandp", bufs=1) as bp, \
                     tc.tile_pool(name="psump", bufs=1,
                                  space=bass.MemorySpace.PSUM) as pp:
                    # static shifted-copy tiles fed by PE shift-matmuls
                    up_s, dn_s = [], []
                    for j in range(2):
                        t_ = bp.tile([ZL, (YB + 2) * X], dt.bfloat16, name=f"ups{j}")
                        nc.vector.memset(t_[:, :], 0.5)
                        up_s.append(t_)
                        t_ = bp.tile([ZL, (YB + 2) * X], dt.bfloat16, name=f"dns{j}")
                        nc.vector.memset(t_[:, :], 0.5)
                        dn_s.append(t_)
                    gu_t = bp.tile([ZL, YL * NW], dt.uint32, name="gut")
                    nc.vector.memset(gu_t[:, :], 0)
                    gd_et = bp.tile([ZL, YL * NW], dt.uint32, name="gdet")
                    nc.vector.memset(gd_et[:, :], ONES)
                    gd_dt = bp.tile([ZL, YL * NW], dt.uint32, name="gddt")
                    nc.vector.memset(gd_dt[:, :], 0)
                    shift_tiles = (up_s, dn_s, None, gu_t, gd_et, gd_dt)
                    A, B = Ea, Eb
                    GA, GB = Ga, Gb
                    for k in [kk for _ in range(KREP) for kk in range(NIT)]:
                        # erode sweep A -> B
                        for bi, (y0, y1) in enumerate(bands(0, YL, YB)):
                            pool_band(A, B, True, y0, y1, aop.min, bp, shift_tiles, bi)
                        # gt erode GA -> GB
                        if "gt" not in KSKIP:
                            gt_pool(GA, GB, True, bp, shift_tiles)
                        # dilate bands of B + delta/skel on owned rows
                        for bi, (y0, y1) in enumerate(bands(0, YO, YB)):
                            D = bp.tile([ZL, (YB + 2) * X], dt.bfloat16, tag="dd")
                            nr = pool_band(B, D, False, y0, y1, aop.max, bp, shift_tiles, bi)
                            rr = min(y1, YO) - y0
                            if "elt" in KSKIP:
                                continue
                            sub = bp.tile([ZL, (YB + 2) * X], dt.bfloat16, tag="t2")
                            nc.vector.tensor_sub(
                                sub[0:ZO, :rr * X], A[0:ZO, y0 * X:(y0 + rr) * X], D[0:ZO, 0:rr * X])
                            if k == 0:
                                nc.scalar.activation(
                                    skel[:, y0 * X:(y0 + rr) * X], sub[0:ZO, :rr * X], AF.Relu)
                            else:
                                tne = bp.tile([ZL, (YB + 2) * X], dt.bfloat16, tag="t1")
                                nc.scalar.activation(
                                    tne[0:ZO, :rr * X], skel[:, y0 * X:(y0 + rr) * X],
                                    AF.Copy, scale=-1.0, bias=1.0)
                                nc.gpsimd.tensor_mul(sub[0:ZO, :rr * X], sub[0:ZO, :rr * X], tne[0:ZO, :rr * X])
                                nc.scalar.activation(tne[0:ZO, :rr * X], sub[0:ZO, :rr * X], AF.Relu)
                                nc.gpsimd.tensor_add(
                                    skel[:, y0 * X:(y0 + rr) * X],
                                    skel[:, y0 * X:(y0 + rr) * X], tne[0:ZO, :rr * X])
                        # gt dilate + delta/skel_gt
                        GD = bp.tile([ZL, YL * NW], dt.uint32, tag="gdl")
                        if "gt" in KSKIP:
                            A, B = B, A
                            GA, GB = GB, GA
                            continue
                        gt_pool(GB, GD, False, bp, shift_tiles)
                        gnt = bp.tile([ZL, YL * NW], dt.uint32, tag="g1")
                        gnt3 = gnt.rearrange("p (r w) -> p r w", w=NW)
                        GD3 = GD.rearrange("p (r w) -> p r w", w=NW)
                        GA3 = GA.rearrange("p (r w) -> p r w", w=NW)
                        skg3 = skg.rearrange("p (r w) -> p r w", w=NW)
                        nc.vector.tensor_single_scalar(gnt3[:, :, 1:7], GD3[:, :, 1:7], ONES, aop.bitwise_xor)
                        nc.vector.tensor_tensor(gnt3[:, :, 1:7], GA3[:, :, 1:7], gnt3[:, :, 1:7], aop.bitwise_and)
                        if k == 0:
                            nc.vector.tensor_copy(skg3[:, :, 1:7], gnt3[0:ZO, 0:YO, 1:7])
                        else:
                            nc.vector.tensor_tensor(
                                skg3[:, :, 1:7], skg3[:, :, 1:7], gnt3[0:ZO, 0:YO, 1:7], aop.bitwise_or)
                        A, B = B, A
                        GA, GB = GB, GA

            # ---------------- final: partial sums --------------------------
            with tc.tile_pool(name="finp", bufs=1) as fp:
                for ci, (y0, y1) in enumerate(bands(0, YO, FCH)):
                    rr = y1 - y0
                    FR = rr * X
                    gtw = fp.tile([ZO, FCH * NW], dt.uint32, tag="fgw")
                    nc.sync.dma_start(gtw[:, :rr * NW], gtb_d[0:ZO, y0:y1, :])
                    mi = fp.tile([ZO, FCH * X], dt.uint32, tag="fmi")
                    mi4 = mi.rearrange("p (r w b) -> p r w b", w=ND, b=32)
                    gw4 = gtw.rearrange("p (r w) -> p r w", w=NW)
                    for b in range(32):
                        nc.vector.tensor_scalar(
                            mi4[:, 0:rr, :, b], gw4[:, 0:rr, 1:7], b, 1,
                            aop.logical_shift_right, aop.bitwise_and)
                    mb = fp.tile([ZO, FCH * X], dt.bfloat16, tag="fmb")
                    nc.vector.tensor_copy(mb[:, :FR], mi[:, :FR])
                    scr = fp.tile([ZO, FCH * X], dt.bfloat16, tag="fsc")
                    # S1 = sum(skel_pred * gt)
                    nc.vector.scalar_tensor_tensor(
                        scr[:, :FR], skel[:, y0 * X:y1 * X], 1.0, mb[:, :FR],
                        aop.mult, aop.mult, accum_out=acc[0:ZO, ci:ci + 1])
                    # S2 = sum(skel_pred)
                    nc.vector.tensor_scalar(
                        scr[:, :FR], skel[:, y0 * X:y1 * X], 0.0, 0.0,
                        aop.add, aop.add, accum_out=acc[0:ZO, 4 + ci:5 + ci])
                    # unpack skel_gt
                    sg4 = skg.rearrange("p (r w) -> p r w", w=NW)
                    for b in range(32):
                        nc.vector.tensor_scalar(
                            mi4[:, 0:rr, :, b], sg4[:, y0:y1, 1:7], b, 1,
                            aop.logical_shift_right, aop.bitwise_and)
                    nc.vector.tensor_copy(mb[:, :FR], mi[:, :FR])
                    # S4 = sum(skel_gt)
                    nc.vector.tensor_scalar(
                        scr[:, :FR], mb[:, :FR], 0.0, 0.0,
                        aop.add, aop.add, accum_out=acc[0:ZO, 12 + ci:13 + ci])
                    # S3 = sum(skel_gt * pred)
                    pt = fp.tile([ZO, FCH * X], dt.bfloat16, tag="fpt")
                    nc.sync.dma_start(pt[:, :FR], p0_d[:, y0 * X:y1 * X])
                    nc.vector.scalar_tensor_tensor(
                        scr[:, :FR], mb[:, :FR], 1.0, pt[:, :FR],
                        aop.mult, aop.mult, accum_out=acc[0:ZO, 8 + ci:9 + ci])
                nc.sync.dma_start(sums_d[:, :], acc[:, :])

    nc.compile()
    return nc


def _host_shard(logits, targets):
    logits = np.ascontiguousarray(np.asarray(logits, dtype=np.float32))
    targets = np.asarray(targets)
    in_maps = []
    for c in range(NCORES):
        b, zh, yh = c >> 2, (c >> 1) & 1, c & 1
        lg = logits[b]
        gt = (targets[b] == 1)
        if zh:
            lg = lg[:, ::-1]
            gt = gt[::-1]
        if yh:
            lg = lg[:, :, ::-1]
            gt = gt[:, ::-1]
        lg = np.ascontiguousarray(lg[:, :ZL, :YL]).astype(_mld.bfloat16)
        gt = np.ascontiguousarray(gt[:ZL, :YL])               # (ZL, YL, X) bool
        words = np.packbits(gt, axis=-1, bitorder="little")   # (ZL, YL, 24) u8
        words = words.view(np.uint32)                         # (ZL, YL, 6)
        gtb = np.zeros((ZL, YL, NW), dtype=np.uint32)
        gtb[:, :, 1:7] = words
        in_maps.append({"lg": lg, "gtb": gtb, "su": _SU, "sd": _SD})
    return in_maps


def kernel(logits, targets):
    from concourse.bass_utils import run_bass_kernel_spmd
    if "nc" not in _CACHE:
        _CACHE["nc"] = _build()
    nc = _CACHE["nc"]
    in_maps = _host_shard(logits, targets)
    res = run_bass_kernel_spmd(nc, in_maps, list(range(NCORES)), trace=False)
    S = np.zeros(4, dtype=np.float64)
    for r in res.results:
        a = r["sums"].astype(np.float64)
        S[0] += a[:, 0:4].sum()
        S[1] += a[:, 4:8].sum()
        S[2] += a[:, 8:12].sum()
        S[3] += a[:, 12:16].sum()
    tprec = (S[0] + 1.0) / (S[1] + 1.0)
    tsens = (S[2] + 1.0) / (S[3] + 1.0)
    cl = 2.0 * tprec * tsens / (tprec + tsens + 1e-7)
    return np.float32(1.0 - cl)



# revision 2
# speedup vs baseline: 4.3654x; 4.3654x over previous
"""CLDice loss Trainium2 kernel: single-core full-loss program.

One core computes all 8 (batch, z-half, y-half) shard problems sequentially
(statically unrolled). The throughput harness round-robins independent loss
computations across 8 disjoint single-core meshes, so the per-call dispatch
overhead is the 1-core floor while all 8 cores stay busy on different
in-flight calls. kernel() runs the same program once on core 0.

Everything ships in ONE packed u8 blob per core (pr u8-quantized sigmoid probs
| bit-packed gt) x NSH shards | su | sd shift matrices. Morphology runs in the
exactly-representable 0..255 bf16 space; gt path is bit-packed u32 boolean
morphology. 12-voxel redundant halos on interior z/y cut sides; lower/right
shards are z/y-flipped on host so the global border is always at index 0.
"""
import os
import numpy as np

NSH = 8                                        # shards per core
NCORES = 8 // NSH                              # cores per loss
Z = Y = X = 192
ZO = YO = 96          # owned extent per quarter
HALO = 12             # 11 erodes + 1 dilate
ZL = YL = ZO + HALO   # local ext extent (108)
NW = 8                # words per row in packed gt (cols 1..6 data, 0/7 pads)
ND = 6                # data words per row
YB = 12               # y-band rows for the float path
NIT = 11              # skeletonize iterations (incl. k=0 init)
FCH = 24              # final-pass row chunk

# per-shard section: pr u8 | gtb u32 ; blob = NSH sections | su | sd
PR_B = ZL * YL * X
GT_B = ZL * YL * NW * 4
SEC_B = PR_B + GT_B
SH_B = ZL * 128 * 2
NBYTES = NSH * SEC_B + 2 * SH_B

_CACHE = {}
_SUMS_SCALE = (255.0, 255.0, 255.0, 1.0)

import ml_dtypes as _mld
_SU = np.zeros((ZL, 128), dtype=_mld.bfloat16)
_SD = np.zeros((ZL, 128), dtype=_mld.bfloat16)
for _m in range(ZL):
    _SU[min(_m + 1, ZL - 1), _m] = 1
    _SD[max(_m - 1, 0), _m] = 1


def _build():
    import concourse.bacc as bacc
    import concourse.bass as bass
    import concourse.mybir as mybir
    from concourse import tile
    from concourse.alu_op_type import AluOpType as aop

    dt = mybir.dt
    AF = mybir.ActivationFunctionType
    nc = bacc.Bacc("TRN2", target_bir_lowering=False, debug=False,
                   num_devices=max(NCORES, 1))

    blob = nc.dram_tensor("blob", [NBYTES], dt.uint8, kind="ExternalInput").ap()
    su_d = blob[NSH * SEC_B:NSH * SEC_B + SH_B].bitcast(dt.bfloat16).rearrange(
        "(z c) -> z c", c=128)
    sd_d = blob[NSH * SEC_B + SH_B:NBYTES].bitcast(dt.bfloat16).rearrange(
        "(z c) -> z c", c=128)
    p0_d = nc.dram_tensor("p0", [NSH, ZO, YO * X], dt.bfloat16).ap()
    sums_d = nc.dram_tensor("sums", [128, NSH * 16], dt.float32,
                            kind="ExternalOutput").ap()

    ONES = 0xFFFFFFFF

    def bands(lo, hi, step):
        return [(y0, min(y0 + step, hi)) for y0 in range(lo, hi, step)]

    KREP = int(os.environ.get("KERNEL_REPEAT", "1"))
    KSKIP = set(os.environ.get("KERNEL_SKIP", "").split(","))

    with tile.TileContext(nc) as tc:
        with tc.tile_pool(name="perm", bufs=1) as perm:
            skel = perm.tile([ZO, YO * X], dt.bfloat16)
            su_t = perm.tile([ZL, 128], dt.bfloat16)
            sd_t = perm.tile([ZL, 128], dt.bfloat16)
            nc.sync.dma_start(su_t[:, :], su_d[:, :])
            nc.sync.dma_start(sd_t[:, :], sd_d[:, :])
            skg = perm.tile([ZO, YO * NW], dt.uint32)
            acc = perm.tile([128, NSH * 16], dt.float32)
            nc.vector.memset(acc[:, :], 0.0)

            with tc.tile_pool(name="epool", bufs=1) as ep, \
                 tc.tile_pool(name="statics", bufs=1) as sp, \
                 tc.tile_pool(name="psump", bufs=1,
                              space=bass.MemorySpace.PSUM) as pp:
                Ea = ep.tile([ZL, YL * X], dt.bfloat16, name="Ea")
                Eb = ep.tile([ZL, YL * X], dt.bfloat16, name="Eb")
                Ga = ep.tile([ZL, YL * NW], dt.uint32, name="Ga")
                Gb = ep.tile([ZL, YL * NW], dt.uint32, name="Gb")

                # static shifted-copy tiles fed by PE shift-matmuls
                up_s, dn_s = [], []
                for j in range(2):
                    t_ = sp.tile([ZL, (YB + 2) * X], dt.bfloat16, name=f"ups{j}")
                    nc.vector.memset(t_[:, :], 0.5)
                    up_s.append(t_)
                    t_ = sp.tile([ZL, (YB + 2) * X], dt.bfloat16, name=f"dns{j}")
                    nc.vector.memset(t_[:, :], 0.5)
                    dn_s.append(t_)
                gu_t = sp.tile([ZL, YL * NW], dt.uint32, name="gut")
                nc.vector.memset(gu_t[:, :], 0)
                gd_t = sp.tile([ZL, YL * NW], dt.uint32, name="gdt")
                nc.vector.memset(gd_t[:, :], 0)
                shift_tiles = (up_s, dn_s, None, gu_t, gd_t, gd_t)

                # ------------ helper: one float 3-tap pool band --------------
                def pool_band(bp, src, dsttile, dst_is_view, y0, y1, op, bidx):
                    """3x3x3 min/max pool of `src` rows [w0,w1) -> rows [y0,r_end)
                    written to dsttile (full [ZL,YL*X] view if dst_is_view else
                    band tile rows local 0..)."""
                    w0, w1 = max(y0 - 1, 0), min(y1 + 1, YL)
                    L = w1 - w0
                    r_end = min(y1, w1 - 1)
                    pe = bidx % 2
                    up = shift_tiles[0][pe]
                    dn = shift_tiles[1][pe]
                    t1 = bp.tile([ZL, (YB + 2) * X], dt.bfloat16, tag=f"t1{pe}")
                    t2 = bp.tile([ZL, (YB + 2) * X], dt.bfloat16, tag=f"t2{pe}")
                    # PE shift-matmuls: up[z] = src[z+1] (clamped), dn[z] = src[z-1]
                    for (mat, dst) in ((su_t, up), (sd_t, dn)):
                        tg = ("psU" if mat is su_t else "psD") + str(pe)
                        for c0 in range(0, L * X, 1024):
                            cw = min(1024, L * X - c0)
                            ps = pp.tile([128, 1024], dt.float32, tag=tg)
                            for j0 in range(0, cw, 512):
                                jw = min(512, cw - j0)
                                nc.tensor.matmul(
                                    ps[:, j0:j0 + jw], mat[:, :],
                                    src[:, w0 * X + c0 + j0:w0 * X + c0 + j0 + jw])
                            nc.scalar.copy(dst[0:ZL, c0:c0 + cw], ps[0:ZL, 0:cw])
                    nc.vector.tensor_tensor(t1[:, :L * X], src[:, w0 * X:w1 * X], up[:, :L * X], op)
                    nc.vector.tensor_tensor(t2[:, :L * X], t1[:, :L * X], dn[:, :L * X], op)
                    # y stage (gpsimd): a_y[j] = op(t2[j], t2[j+1]) j in 0..L-2
                    t3 = bp.tile([ZL, (YB + 2) * X], dt.bfloat16, tag=f"t1{pe}")
                    nc.vector.tensor_tensor(t3[:, :(L - 1) * X], t2[:, 0:(L - 1) * X], t2[:, X:L * X], op)
                    # out rows [y0, r_end): out[r] = op(a_y[r-1-w0], a_y[r-w0]); r==0 -> a_y[0]
                    t4 = bp.tile([ZL, (YB + 2) * X], dt.bfloat16, tag=f"t2{pe}")
                    nr = 0
                    if y0 == 0:
                        nc.vector.tensor_copy(t4[:, 0:X], t3[:, 0:X])
                        if r_end > 1:
                            nc.vector.tensor_tensor(
                                t4[:, X:r_end * X], t3[:, 0:(r_end - 1) * X], t3[:, X:r_end * X], op)
                        nr = r_end
                    else:
                        nr = r_end - y0
                        j0 = y0 - w0  # == 1
                        nc.vector.tensor_tensor(
                            t4[:, 0:nr * X],
                            t3[:, (j0 - 1) * X:(j0 - 1 + nr) * X],
                            t3[:, j0 * X:(j0 + nr) * X], op)
                    # x stage on t4 rows 0..nr
                    t43 = t4.rearrange("p (r c) -> p r c", c=X)
                    t5 = bp.tile([ZL, (YB + 2) * X], dt.bfloat16, tag=f"t1{pe}")
                    t53 = t5.rearrange("p (r c) -> p r c", c=X)
                    nc.vector.tensor_tensor(t53[:, 0:nr, 0:X - 1], t43[:, 0:nr, 0:X - 1], t43[:, 0:nr, 1:X], op)
                    nc.vector.tensor_copy(t53[:, 0:nr, X - 1:X], t43[:, 0:nr, X - 1:X])
                    d3 = dsttile.rearrange("p (r c) -> p r c", c=X)
                    o0, o1 = (y0, y0 + nr) if dst_is_view else (0, nr)
                    nc.vector.tensor_tensor(d3[:, o0:o1, 1:X], t53[:, 0:nr, 0:X - 1], t53[:, 0:nr, 1:X], op)
                    nc.vector.tensor_copy(d3[:, o0:o1, 0:1], t53[:, 0:nr, 0:1])
                    if dst_is_view and r_end < y1:
                        # fill clipped halo-edge rows with bounded values
                        nc.vector.tensor_copy(d3[:, r_end:y1, :], t53[:, nr - 1:nr, :])
                    return nr

                # ------------ helper: packed-gt 3-tap pool --------------------
                def gt_pool(bp, src, dst, op_and):
                    """3x3x3 AND(erode)/OR(dilate) of packed src -> dst."""
                    top = aop.bitwise_and if op_and else aop.bitwise_or
                    # pads on src: ones for erode, zeros for dilate
                    s3 = src.rearrange("p (r w) -> p r w", w=NW)
                    nc.vector.memset(s3[:, :, 0:1], ONES if op_and else 0)
                    nc.vector.memset(s3[:, :, 7:8], ONES if op_and else 0)
                    FW = YL * NW
                    gu = shift_tiles[3]
                    gd = shift_tiles[4]
                    # row 0 of gd is the global-border pad: AND-identity for
                    # erode, OR-identity for dilate
                    nc.vector.memset(gd[0:1, :], ONES if op_and else 0)
                    nc.scalar.dma_start(gu[0:ZL - 1, :], src[1:ZL, :])
                    nc.scalar.dma_start(gd[1:ZL, :], src[0:ZL - 1, :])
                    g1 = bp.tile([ZL, FW], dt.uint32, tag="g1")
                    g2 = bp.tile([ZL, FW], dt.uint32, tag="g2")
                    nc.vector.tensor_tensor(g1[:, :], src[:, :], gu[:, :], top)
                    nc.vector.tensor_tensor(g2[:, :], g1[:, :], gd[:, :], top)
                    # y stage
                    g13 = g1.rearrange("p (r w) -> p r w", w=NW)
                    g23 = g2.rearrange("p (r w) -> p r w", w=NW)
                    nc.vector.tensor_tensor(g13[:, 0:YL - 1, :], g23[:, 0:YL - 1, :], g23[:, 1:YL, :], top)
                    g3 = bp.tile([ZL, FW], dt.uint32, tag="g3")
                    g33 = g3.rearrange("p (r w) -> p r w", w=NW)
                    nc.vector.tensor_copy(g33[:, 0:1, :], g13[:, 0:1, :])
                    nc.vector.tensor_tensor(g33[:, 1:YL - 1, :], g13[:, 0:YL - 2, :], g13[:, 1:YL - 1, :], top)
                    nc.vector.tensor_copy(g33[:, YL - 1:YL, :], g13[:, YL - 2:YL - 1, :])
                    # x stage (bits, little endian: value(x+1) of bit b is bit b+1)
                    d3 = dst.rearrange("p (r w) -> p r w", w=NW)
                    s1 = bp.tile([ZL, FW], dt.uint32, tag="g2")
                    s13 = s1.rearrange("p (r w) -> p r w", w=NW)
                    s2 = bp.tile([ZL, FW], dt.uint32, tag="gu")
                    s23 = s2.rearrange("p (r w) -> p r w", w=NW)
                    # t_minus = (w<<1)|(w_prev>>31) ; t_plus = (w>>1)|(w_next<<31)
                    nc.vector.tensor_single_scalar(s13[:, :, 1:7], g33[:, :, 1:7], 1, aop.logical_shift_left)
                    nc.vector.tensor_single_scalar(s23[:, :, 1:7], g33[:, :, 0:6], 31, aop.logical_shift_right)
                    nc.vector.tensor_tensor(s13[:, :, 1:7], s13[:, :, 1:7], s23[:, :, 1:7], aop.bitwise_or)
                    nc.vector.tensor_tensor(s13[:, :, 1:7], s13[:, :, 1:7], g33[:, :, 1:7], top)
                    nc.vector.tensor_single_scalar(s23[:, :, 1:7], g33[:, :, 1:7], 1, aop.logical_shift_right)
                    nc.vector.tensor_tensor(d3[:, :, 1:7], s13[:, :, 1:7], s23[:, :, 1:7], top)
                    nc.vector.tensor_single_scalar(s23[:, :, 1:7], g33[:, :, 2:8], 31, aop.logical_shift_left)
                    nc.vector.tensor_tensor(d3[:, :, 1:7], d3[:, :, 1:7], s23[:, :, 1:7], top)

                # ================== per-shard unrolled ========================
                for sh in range(NSH):
                    pr_d = blob[sh * SEC_B:sh * SEC_B + PR_B].rearrange(
                        "(z y x) -> z y x", z=ZL, y=YL)
                    gtb_d = blob[sh * SEC_B + PR_B:(sh + 1) * SEC_B].bitcast(
                        dt.uint32).rearrange("(z y w) -> z y w", z=ZL, y=YL)

                    with tc.tile_pool(name=f"bandp{sh}", bufs=1) as bp:
                        # ---- init: u8 pred -> bf16 (0..255), load gt bits ----
                        nc.sync.dma_start(Ga[:, :], gtb_d[:, :, :])
                        for (y0, y1) in bands(0, YL, YB):
                            rr = y1 - y0
                            c0 = bp.tile([ZL, YB * X], dt.uint8, tag="ic0")
                            nc.sync.dma_start(c0[:, :rr * X], pr_d[:, y0:y1, :])
                            nc.vector.tensor_copy(Ea[:, y0 * X:y1 * X], c0[:, :rr * X])
                        nc.sync.dma_start(p0_d[sh, :, :], Ea[0:ZO, 0:YO * X])

                        # ---- main iterations ----
                        A, B = Ea, Eb
                        GA, GB = Ga, Gb
                        for k in [kk for _ in range(KREP) for kk in range(NIT)]:
                            # erode sweep A -> B
                            for bi, (y0, y1) in enumerate(bands(0, YL, YB)):
                                pool_band(bp, A, B, True, y0, y1, aop.min, bi)
                            # gt erode GA -> GB
                            if "gt" not in KSKIP:
                                gt_pool(bp, GA, GB, True)
                            # dilate bands of B + delta/skel on owned rows
                            for bi, (y0, y1) in enumerate(bands(0, YO, YB)):
                                D = bp.tile([ZL, (YB + 2) * X], dt.bfloat16, tag=f"dd{bi % 2}")
                                nr = pool_band(bp, B, D, False, y0, y1, aop.max, bi)
                                rr = min(y1, YO) - y0
                                if "elt" in KSKIP:
                                    continue
                                sub = bp.tile([ZL, (YB + 2) * X], dt.bfloat16, tag=f"t2{bi % 2}")
                                nc.vector.tensor_sub(
                                    sub[0:ZO, :rr * X], A[0:ZO, y0 * X:(y0 + rr) * X], D[0:ZO, 0:rr * X])
                                if k == 0:
                                    nc.scalar.activation(
                                        skel[:, y0 * X:(y0 + rr) * X], sub[0:ZO, :rr * X], AF.Relu)
                                else:
                                    tne = bp.tile([ZL, (YB + 2) * X], dt.bfloat16, tag=f"t1{bi % 2}")
                                    # tne = 1 - skel/255 (skel lives in 0..255 space)
                                    nc.scalar.activation(
                                        tne[0:ZO, :rr * X], skel[:, y0 * X:(y0 + rr) * X],
                                        AF.Copy, scale=-1.0 / 255.0, bias=1.0)
                                    nc.gpsimd.tensor_mul(sub[0:ZO, :rr * X], sub[0:ZO, :rr * X], tne[0:ZO, :rr * X])
                                    nc.scalar.activation(tne[0:ZO, :rr * X], sub[0:ZO, :rr * X], AF.Relu)
                                    nc.gpsimd.tensor_add(
                                        skel[:, y0 * X:(y0 + rr) * X],
                                        skel[:, y0 * X:(y0 + rr) * X], tne[0:ZO, :rr * X])
                            # gt dilate + delta/skel_gt
                            GD = bp.tile([ZL, YL * NW], dt.uint32, tag="gdl")
                            if "gt" in KSKIP:
                                A, B = B, A
                                GA, GB = GB, GA
                                continue
                            gt_pool(bp, GB, GD, False)
                            gnt = bp.tile([ZL, YL * NW], dt.uint32, tag="g1")
                            gnt3 = gnt.rearrange("p (r w) -> p r w", w=NW)
                            GD3 = GD.rearrange("p (r w) -> p r w", w=NW)
                            GA3 = GA.rearrange("p (r w) -> p r w", w=NW)
                            skg3 = skg.rearrange("p (r w) -> p r w", w=NW)
                            nc.vector.tensor_single_scalar(gnt3[:, :, 1:7], GD3[:, :, 1:7], ONES, aop.bitwise_xor)
                            nc.vector.tensor_tensor(gnt3[:, :, 1:7], GA3[:, :, 1:7], gnt3[:, :, 1:7], aop.bitwise_and)
                            if k == 0:
                                nc.vector.tensor_copy(skg3[:, :, 1:7], gnt3[0:ZO, 0:YO, 1:7])
                            else:
                                nc.vector.tensor_tensor(
                                    skg3[:, :, 1:7], skg3[:, :, 1:7], gnt3[0:ZO, 0:YO, 1:7], aop.bitwise_or)
                            A, B = B, A
                            GA, GB = GB, GA

                    # ---- final: partial sums for this shard ----
                    with tc.tile_pool(name=f"finp{sh}", bufs=1) as fp:
                        for ci, (y0, y1) in enumerate(
                                [] if "fin" in KSKIP else bands(0, YO, FCH)):
                            rr = y1 - y0
                            FR = rr * X
                            co = sh * 16
                            gtw = fp.tile([ZO, FCH * NW], dt.uint32, tag="fgw")
                            nc.sync.dma_start(gtw[:, :rr * NW], gtb_d[0:ZO, y0:y1, :])
                            mi = fp.tile([ZO, FCH * X], dt.uint32, tag="fmi")
                            mi4 = mi.rearrange("p (r w b) -> p r w b", w=ND, b=32)
                            gw4 = gtw.rearrange("p (r w) -> p r w", w=NW)
                            for b in range(32):
                                nc.vector.tensor_scalar(
                                    mi4[:, 0:rr, :, b], gw4[:, 0:rr, 1:7], b, 1,
                                    aop.logical_shift_right, aop.bitwise_and)
                            mb = fp.tile([ZO, FCH * X], dt.bfloat16, tag="fmb")
                            nc.vector.tensor_copy(mb[:, :FR], mi[:, :FR])
                            scr = fp.tile([ZO, FCH * X], dt.bfloat16, tag="fsc")
                            # S1 = sum(skel_pred * gt)   (0..255-scaled)
                            nc.vector.scalar_tensor_tensor(
                                scr[:, :FR], skel[:, y0 * X:y1 * X], 1.0, mb[:, :FR],
                                aop.mult, aop.mult, accum_out=acc[0:ZO, co + ci:co + ci + 1])
                            # S2 = sum(skel_pred)        (0..255-scaled)
                            nc.vector.tensor_scalar(
                                scr[:, :FR], skel[:, y0 * X:y1 * X], 0.0, 0.0,
                                aop.add, aop.add, accum_out=acc[0:ZO, co + 4 + ci:co + 5 + ci])
                            # unpack skel_gt
                            sg4 = skg.rearrange("p (r w) -> p r w", w=NW)
                            for b in range(32):
                                nc.vector.tensor_scalar(
                                    mi4[:, 0:rr, :, b], sg4[:, y0:y1, 1:7], b, 1,
                                    aop.logical_shift_right, aop.bitwise_and)
                            nc.vector.tensor_copy(mb[:, :FR], mi[:, :FR])
                            # S4 = sum(skel_gt)
                            nc.vector.tensor_scalar(
                                scr[:, :FR], mb[:, :FR], 0.0, 0.0,
                                aop.add, aop.add, accum_out=acc[0:ZO, co + 12 + ci:co + 13 + ci])
                            # S3 = sum(skel_gt * pred)   (0..255-scaled)
                            pt = fp.tile([ZO, FCH * X], dt.bfloat16, tag="fpt")
                            nc.sync.dma_start(pt[:, :FR], p0_d[sh, :, y0 * X:y1 * X])
                            nc.vector.scalar_tensor_tensor(
                                scr[:, :FR], mb[:, :FR], 1.0, pt[:, :FR],
                                aop.mult, aop.mult, accum_out=acc[0:ZO, co + 8 + ci:co + 9 + ci])

            nc.sync.dma_start(sums_d[:, :], acc[:, :])

    nc.compile()
    return nc


def _host_shard(logits, targets):
    logits = np.asarray(logits, dtype=np.float32)
    targets = np.asarray(targets)
    # quantized sigmoid probs per batch, computed once
    q_all = []
    for b in range(2):
        d = logits[b, 1] - logits[b, 0]
        p = 0.5 * (1.0 + np.tanh(0.5 * d))
        q_all.append(np.rint(p * 255.0).astype(np.uint8))
    secs = []
    for c in range(8):
        b, zh, yh = c >> 2, (c >> 1) & 1, c & 1
        pr = q_all[b]
        gt = (targets[b] == 1)
        if zh:
            pr = pr[::-1]
            gt = gt[::-1]
        if yh:
            pr = pr[:, ::-1]
            gt = gt[:, ::-1]
        pr = np.ascontiguousarray(pr[:ZL, :YL])
        gt = np.ascontiguousarray(gt[:ZL, :YL])              # (ZL, YL, X) bool
        words = np.packbits(gt, axis=-1, bitorder="little")  # (ZL, YL, 24) u8
        words = words.view(np.uint32)                        # (ZL, YL, 6)
        gtb = np.zeros((ZL, YL, NW), dtype=np.uint32)
        gtb[:, :, 1:7] = words
        secs.append(np.concatenate([pr.ravel(), gtb.view(np.uint8).ravel()]))
    shw = [_SU.view(np.uint8).ravel(), _SD.view(np.uint8).ravel()]
    in_maps = []
    for core in range(NCORES):
        b8 = np.concatenate(secs[core * NSH:(core + 1) * NSH] + shw)
        in_maps.append({"blob": b8})
    return in_maps


def _loss_from_sums(sums_list):
    S = np.zeros(4, dtype=np.float64)
    for a in sums_list:
        a = a.astype(np.float64).reshape(128, -1, 16)
        S[0] += a[:, :, 0:4].sum()
        S[1] += a[:, :, 4:8].sum()
        S[2] += a[:, :, 8:12].sum()
        S[3] += a[:, :, 12:16].sum()
    S[0] /= 255.0
    S[1] /= 255.0
    S[2] /= 255.0
    tprec = (S[0] + 1.0) / (S[1] + 1.0)
    tsens = (S[2] + 1.0) / (S[3] + 1.0)
    cl = 2.0 * tprec * tsens / (tprec + tsens + 1e-7)
    return np.float32(1.0 - cl)


def kernel(logits, targets):
    from concourse.bass_utils import run_bass_kernel_spmd
    if "nc" not in _CACHE:
        _CACHE["nc"] = _build()
    nc = _CACHE["nc"]
    in_maps = _host_shard(logits, targets)
    res = run_bass_kernel_spmd(nc, in_maps, list(range(NCORES)), trace=False)
    return _loss_from_sums([r["sums"] for r in res.results])
